# revision 12
# baseline (speedup 1.0000x reference)
"""Trainium2 Bass kernel for nn_CrossScalePeriodicFeatureAggregator.

Reference computation (per expert e with patch size p_e, L_e = 336 / p_e):
    h = einsum('nld,pd->nlp', xs_e, W_e) + b_e      # [128, L_e, p_e*512]
    h -> reshape [128, 336, 512]                     # seq-stitch
    proj = h @ Wp.T + bp                             # shared projection
    out[batch_index] += gate * proj                  # gated scatter-combine

Kernel strategy (8 cores, SPMD):
  * Algebraic fusion: the chained matmuls collapse into one. For output
    position s = l*p_e + q:  out[n, s, :] = x[n, l, :] @ WF_e[q]  where
    WF_e[q] = W_e[q*512:(q+1)*512, :].T @ Wp.T   (precomputed on host).
    Halves device FLOPs (90 GF instead of 180 GF). Gates fold into x rows.
  * Expert-parallel q-split sharding: 2 cores per expert, each owning half
    of that expert's patch offsets q over ALL 128 rows. Per-core weights
    shrink to <= 6 MB bf16 and stay RESIDENT in SBUF (48 KiB/partition,
    loaded once), so steady state has zero weight DMA; compute is perfectly
    balanced (21504 tokens/core). x (<= 10.5 MB bf16) is SBUF-resident too.
  * bf16 weights/activations/outputs (fp32 PSUM accumulation): halves all
    HBM traffic and enables FWL (fast weight load) on the PE array, which
    fp32r does not get. Measured rel-err is ~3e-3, inside the 2e-2 gate.
  * Weights-stationary matmuls: lhsT = WF chunk [k=128, dout=128], moving
    rhs = x tokens. PSUM tile [dout 128, tok 448], k-accumulated over 4
    chunks. k-OUTER ordering (4 consecutive MMs share one stationary) is
    ~3x faster than k-inner: LDWEIGHTS amortizes and pulls ahead.
    768 MMs/pass = 344k PE cycles = 143 us floor at 2.4 GHz; measured
    ~142 us/pass = at the bf16 roofline (fp8 fails the accuracy gate).
  * Uniform SPMD program: 12 segments x 1792 tokens. Per-core differences
    live entirely in DRAM *content* (x token slots tiled cyclically,
    per-segment weight table), never in the instruction stream.
  * PSUM evictions (fp32 -> bf16 cast) alternate DVE/ACT; output stores
    (21 MB/pass, the only steady-state DMA) ride GpSimdE's ring.
"""
import numpy as np

PATCH = [4, 8, 12, 24]
SEQ = 336
D = 512
NE = 4
BATCH = 256
ROWS = 128                                  # rows per expert (all on core)
N_CORES = 8
KC = 4                                      # contraction chunks of 128
L = [SEQ // p for p in PATCH]               # [84, 42, 28, 14]
TOK = [ROWS * l for l in L]                 # expert tokens: [10752, 5376, 3584, 1792]
NSEG = 12                                   # segments per core per iteration
SEGTOK = 1792                               # tokens per segment (14 tiles of 128)
NSLOT = 6                                   # x-buffer slots (6*1792 = 10752 tokens)
SLOTTOK = NSLOT * SEGTOK                    # 10752
NCHUNK = 4                                  # token chunks per segment
CHTOK = SEGTOK // NCHUNK                    # 448
NDB = 4                                     # dout blocks of 128
UNROLL = 16                                 # bodies per For_i iteration (timing)

CORE_EXPERT = [0, 0, 1, 1, 2, 2, 3, 3]
JOBS = [2, 4, 6, 12]                        # q's per core for that expert
SEG_PER_JOB = [NSEG // j for j in JOBS]     # [6, 3, 2, 1]
NSLOT_E = [TOK[e] // SEGTOK for e in range(NE)]   # [6, 3, 2, 1]

_CACHED = {}


def _bf16():
    import ml_dtypes
    return np.dtype(ml_dtypes.bfloat16)


def _build_nc(loop_n=0, internal_wf=False, internal_out=False,
              internal_x=False):
    """loop_n>0 wraps the compute body in a hardware For_i loop (differential
    HW timing); internal_wf/internal_out source weights from / sink outputs to
    internal DRAM and internal_x replaces the x upload with a device memset,
    so timing builds transfer (almost) nothing to/from the host."""
    import concourse.mybir as mybir
    from concourse import bacc
    from concourse.tile import TileContext

    bf16 = mybir.dt.bfloat16
    f32 = mybir.dt.float32

    nc = bacc.Bacc("TRN2", target_bir_lowering=False, debug=False,
                   num_devices=N_CORES)
    xt = wf = None
    if not internal_x:
        xt = nc.dram_tensor("xt", [128, KC * SLOTTOK], bf16,
                            kind="ExternalInput")
        if internal_wf:
            wf = nc.dram_tensor("iwf", [NSEG, 128, KC * D], bf16)
        else:
            wf = nc.dram_tensor("wf", [NSEG, 128, KC * D], bf16,
                                kind="ExternalInput")
    # out[s][p, db*SEGTOK + t] = proj value for dout = db*128 + p of segment
    # token t. Fully contiguous per partition -> one large DMA per segment.
    if internal_out:
        out = nc.dram_tensor("iout", [NSEG, 128, NDB * SEGTOK], bf16)
        tiny = nc.dram_tensor("tiny", [128, D], bf16, kind="ExternalOutput")
    else:
        out = nc.dram_tensor("out", [NSEG, 128, NDB * SEGTOK], bf16,
                             kind="ExternalOutput")

    with TileContext(nc) as tc:
        with (
            tc.tile_pool(name="xpool", bufs=1) as xpool,
            tc.tile_pool(name="spool", bufs=3) as spool,
            tc.tile_pool(name="ppool", bufs=8, space="PSUM") as ppool,
        ):
            xtile = xpool.tile([128, KC * SLOTTOK], bf16, tag="xt")
            wtile = xpool.tile([128, NSEG * KC * D], bf16, tag="wt")
            if internal_x:
                # Timing builds: no host upload at all — data values don't
                # affect engine timing (no data-dependent control flow).
                nc.vector.memset(xtile[:], 0.001)
                nc.vector.memset(wtile[:], 0.001)
            else:
                # 21 KiB per-partition chunks (descriptor limit is 64 KiB)
                for k in range(KC):
                    nc.sync.dma_start(
                        xtile[:, k * SLOTTOK:(k + 1) * SLOTTOK],
                        xt.ap()[:, k * SLOTTOK:(k + 1) * SLOTTOK])
                # All 12 segment weight tiles stay resident in SBUF (48 KiB
                # per partition): loaded once, zero weight DMA in steady
                # state.
                for s in range(NSEG):
                    nc.sync.dma_start(
                        wtile[:, s * KC * D:(s + 1) * KC * D], wf.ap()[s])

            state = {"flip": 0}

            def body():
                for s in range(NSEG):
                    st = spool.tile([128, NDB * SEGTOK], bf16, tag="st")
                    pos = s % NSLOT
                    xoff = [k * SLOTTOK + pos * SEGTOK for k in range(KC)]
                    woff = s * KC * D
                    for db in range(NDB):
                        # k-outer / c-inner: 4 consecutive MMs share one
                        # stationary -> LDWEIGHTS amortized + pulled ahead
                        # (measured ~3x faster than k-inner ordering).
                        ptiles = [ppool.tile([128, D], f32, name="ps")
                                  for _ in range(NCHUNK)]
                        for k in range(KC):
                            for c in range(NCHUNK):
                                nc.tensor.matmul(
                                    ptiles[c][:, :CHTOK],
                                    wtile[:, woff + k * D + db * 128:
                                          woff + k * D + (db + 1) * 128],
                                    xtile[:, xoff[k] + c * CHTOK:
                                          xoff[k] + (c + 1) * CHTOK],
                                    start=(k == 0), stop=(k == KC - 1),
                                )
                        for c in range(NCHUNK):
                            dst = st[:, db * SEGTOK + c * CHTOK:
                                     db * SEGTOK + (c + 1) * CHTOK]
                            if state["flip"] % 2:
                                nc.scalar.copy(dst, ptiles[c][:, :CHTOK])
                            else:
                                nc.vector.tensor_copy(dst, ptiles[c][:, :CHTOK])
                            state["flip"] += 1
                    nc.gpsimd.dma_start(out.ap()[s], st[:])

            if loop_n > 0:
                # 4x unrolled: the For_i back-edge is an all-engine barrier
                # (drains the last store DMA + re-throttles HAM); amortizing
                # it over 4 bodies measured ~16 us/pass faster.
                assert loop_n % UNROLL == 0
                with tc.For_i(0, loop_n // UNROLL, 1):
                    for _ in range(UNROLL):
                        body()
            else:
                body()
            if internal_out:
                nc.sync.dma_start(tiny.ap(), xtile[:, :D])
    nc.compile()
    return nc


def _get_nc():
    if "nc" not in _CACHED:
        _CACHED["nc"] = _build_nc()
    return _CACHED["nc"]


def _prep(xs, Ws, gates, Wp, batch_index, expert_index):
    """Host-side shard prep. Returns (in_maps, row_of_expert, g_row)."""
    bf16 = _bf16()
    row_of_expert = [np.nonzero(expert_index == e)[0] for e in range(NE)]
    g_row = gates[batch_index, expert_index].astype(np.float32)   # [NNZ]

    # Fused weights WF_e[q] = W_e[q*512:(q+1)*512, :].T @ Wp.T -> [c, d_out];
    # device layout wf_e[q, p, k*512+d] with c = 128k + p.
    wf_dev = []
    for e in range(NE):
        p = PATCH[e]
        w = Ws[e].reshape(p, D, D)                     # [q, d_mid, c]
        WF = np.einsum("qdc,od->qco", w, Wp, optimize=True)   # [q, c, d_out]
        wf_dev.append(np.ascontiguousarray(
            WF.reshape(p, KC, 128, D).transpose(0, 2, 1, 3)   # [q, p128, k, d]
              .reshape(p, 128, KC * D)).astype(bf16))

    # Gate-scaled token streams per expert, device layout
    # xr_e[p, k, t] = x_tokens[t, 128k + p], then tiled to SLOTTOK tokens.
    x_dev = []
    for e in range(NE):
        rows = row_of_expert[e]
        gr = g_row[rows]
        x = (xs[e] * gr[:, None, None]).reshape(TOK[e], D)
        xr = x.reshape(TOK[e], KC, 128).transpose(2, 1, 0)     # [128, k, T]
        xr = np.tile(xr, (1, 1, SLOTTOK // TOK[e]))            # [128, k, 10752]
        x_dev.append(np.ascontiguousarray(
            xr.reshape(128, KC * SLOTTOK)).astype(bf16))

    in_maps = []
    for c in range(N_CORES):
        e = CORE_EXPERT[c]
        q0 = (c % 2) * JOBS[e]
        qidx = [q0 + s // SEG_PER_JOB[e] for s in range(NSEG)]
        in_maps.append({
            "xt": x_dev[e],
            "wf": np.ascontiguousarray(wf_dev[e][qidx]),       # [12, 128, 2048]
        })
    return in_maps, row_of_expert, g_row


def _combine(results, row_of_expert, batch_index):
    """Reassemble per-segment device outputs and gated-combine per batch."""
    combined = np.zeros((BATCH, SEQ, D), np.float32)
    for e in range(NE):
        p = PATCH[e]
        # acc[token_flat, q, dout]; token_flat = n*L_e + l
        acc = np.zeros((TOK[e], p, D), np.float32)
        for c in range(N_CORES):
            if CORE_EXPERT[c] != e:
                continue
            q0 = (c % 2) * JOBS[e]
            # [s, p128, db, t] -> [s, t, db, p128] -> [s, t, dout]
            arr = np.asarray(results[c]["out"]).astype(np.float32)
            arr = arr.reshape(NSEG, 128, NDB, SEGTOK).transpose(0, 3, 2, 1)
            arr = arr.reshape(NSEG, SEGTOK, D)
            for s in range(NSEG):
                q = q0 + s // SEG_PER_JOB[e]
                slot = (s % NSLOT) % NSLOT_E[e]
                acc[slot * SEGTOK:(slot + 1) * SEGTOK, q, :] = arr[s]
        # [n, l, q, dout] -> [n, l*p + q, dout]
        full = acc.reshape(ROWS, L[e], p, D).reshape(ROWS, SEQ, D)
        bids = batch_index[row_of_expert[e]]
        if len(np.unique(bids)) == len(bids):
            combined[bids] += full
        else:
            np.add.at(combined, bids, full)
    return combined


def kernel(xs0, xs1, xs2, xs3, gates, W0, b0, W1, b1, W2, b2, W3, b3, Wp, bp,
           batch_index, expert_index):
    from concourse.bass_utils import run_bass_kernel_spmd

    xs = [np.asarray(x, np.float32) for x in (xs0, xs1, xs2, xs3)]
    Ws = [np.asarray(w, np.float32) for w in (W0, W1, W2, W3)]
    bs = [np.asarray(b, np.float32) for b in (b0, b1, b2, b3)]
    gates = np.asarray(gates, np.float32)
    Wp = np.asarray(Wp, np.float32)
    bp = np.asarray(bp, np.float32)
    batch_index = np.asarray(batch_index)
    expert_index = np.asarray(expert_index)

    in_maps, row_of_expert, g_row = _prep(xs, Ws, gates, Wp,
                                          batch_index, expert_index)
    nc = _get_nc()
    res = run_bass_kernel_spmd(nc, in_maps, list(range(N_CORES)))

    combined = _combine(res.results, row_of_expert, batch_index)

    # Bias terms (zero in this problem's inputs; handled for correctness).
    if any(np.any(b) for b in bs) or np.any(bp):
        for e in range(NE):
            p = PATCH[e]
            bF = bs[e].reshape(p, D) @ Wp.T + bp       # [q, d_out]
            bias_seq = np.tile(bF, (L[e], 1)).reshape(SEQ, D)
            bids = batch_index[row_of_expert[e]]
            gr = g_row[row_of_expert[e]]
            contrib = gr[:, None, None] * bias_seq[None]
            if len(np.unique(bids)) == len(bids):
                combined[bids] += contrib
            else:
                np.add.at(combined, bids, contrib)

    return combined
